# revision 6
# baseline (speedup 1.0000x reference)
"""Trainium2 Bass kernel for nn_Attention (B=4, S=2048, D=2048, H=16, KV=4, HD=128).

Sharding (8 cores): data-parallel over batch (4) x tensor-parallel over
KV-head-group halves (2). Core c handles batch b=c//2 and q-heads
[8*(c%2), 8*(c%2)+8) == kv groups {2*(c%2), 2*(c%2)+1}. Each core produces a
partial output (its heads' contribution through wo); the host sums the two
partials per batch.

All big matmuls run in float32r (full PE speed, ~1.6e-4 rel err). Flash-style
attention: scores (q stationary, kT moving) -> exp on ACT with fused scale and
accumulated row sums (no max subtraction; scores are O(6) here) -> per-128-block
PE transpose with diag(1/denom) as the transpose multiplicand (normalization for
free) -> AV accumulated in PSUM (V stationary, probsT moving) -> output
projection (woT stationary, attT moving) into a transposed partial output;
host transposes back and sums core pairs.
"""
import numpy as np

B, S, D = 4, 2048, 2048
H, KV, HD = 16, 4, 128
NREP = H // KV
SCALE = float(HD) ** -0.5

SB = S // 128          # 16 s-blocks
KT = D // 128          # 16 contraction tiles for projections
QSB = S // 512         # 4 q-superblocks
HPC = 8                # q heads per core
GPC = 2                # kv groups per core

_compiled = {}


def _build(causal: bool):
    import concourse.bass as bass  # noqa: F401
    import concourse.tile as tile
    from concourse import bacc, mybir
    from concourse.masks import make_identity

    f32 = mybir.dt.float32
    f32r = mybir.dt.float32r
    AF = mybir.ActivationFunctionType
    ALU = mybir.AluOpType

    nc = bacc.Bacc("TRN2")

    xT = nc.dram_tensor("xT", [D, S], f32r, kind="ExternalInput")
    wqT = nc.dram_tensor("wqT", [D, HPC * HD], f32r, kind="ExternalInput")
    wkT = nc.dram_tensor("wkT", [D, GPC * HD], f32r, kind="ExternalInput")
    wvT = nc.dram_tensor("wvT", [D, GPC * HD], f32r, kind="ExternalInput")
    woT = nc.dram_tensor("woT", [HPC * HD, D], f32r, kind="ExternalInput")
    cosS = nc.dram_tensor("cosS", [128, SB, 64], f32, kind="ExternalInput")
    sinS = nc.dram_tensor("sinS", [128, SB, 64], f32, kind="ExternalInput")
    mtile = nc.dram_tensor("mtile", [128, 128], f32, kind="ExternalInput")
    outT = nc.dram_tensor("outT", [D, S], f32, kind="ExternalOutput")

    xT3 = xT.rearrange("(kt p) s -> p kt s", p=128)
    woT3 = woT.rearrange("(h p) d -> p h d", p=128)

    with tile.TileContext(nc) as tc:
        with tc.tile_pool(name="persist", bufs=1) as persist:
            qT = [persist.tile([128, S], f32r, tag=f"qT{h}", name=f"qT{h}") for h in range(HPC)]
            kT = [persist.tile([128, S], f32r, tag=f"kTg{g}", name=f"kTg{g}") for g in range(GPC)]
            vsb = [persist.tile([128, SB, 128], f32r, tag=f"v{g}", name=f"v{g}") for g in range(GPC)]
            ident_f = persist.tile([128, 128], f32, tag="identf")
            make_identity(nc, ident_f)
            ident = persist.tile([128, 128], f32r, tag="ident")
            nc.vector.tensor_copy(out=ident, in_=ident_f)
            msk = persist.tile([128, 128], f32, tag="msk")
            nc.sync.dma_start(out=msk, in_=mtile[:, :])
            cos_t = persist.tile([128, SB, 64], f32, tag="cos")
            sin_t = persist.tile([128, SB, 64], f32, tag="sin")
            nc.sync.dma_start(out=cos_t, in_=cosS[:, :, :])
            nc.sync.dma_start(out=sin_t, in_=sinS[:, :, :])

            # ------------ Stage 1: projections + RoPE + transposes ----------
            def proj_pass(wT_ap, e_width, kind, head_base=0):
                nh = e_width // 128
                with tc.tile_pool(name="w1", bufs=1) as wpool, \
                     tc.tile_pool(name="xs1", bufs=2) as xpool, \
                     tc.tile_pool(name="rs1", bufs=2) as rpool, \
                     tc.tile_pool(name="pq1", bufs=2, space="PSUM") as pqp, \
                     tc.tile_pool(name="pt1", bufs=2, space="PSUM") as ptp:
                    wt = wpool.tile([128, KT, e_width], f32r, tag="wt")
                    nc.sync.dma_start(
                        out=wt, in_=wT_ap.rearrange("(kt p) e -> p kt e", p=128))
                    for sb in range(SB):
                        xs = xpool.tile([128, KT, 128], f32r, tag="xs")
                        nc.sync.dma_start(
                            out=xs, in_=xT3[:, :, sb * 128:(sb + 1) * 128])
                        ps = pqp.tile([128, e_width], f32, tag="ps")
                        for kt in range(KT):
                            nc.tensor.matmul(
                                ps, xs[:, kt, :], wt[:, kt, :],
                                start=(kt == 0), stop=(kt == KT - 1))
                        ps3 = ps.rearrange("p (h d) -> p h d", d=128)
                        if kind == "v":
                            for h in range(nh):
                                nc.scalar.copy(
                                    out=vsb[head_base + h][:, sb, :],
                                    in_=ps3[:, h, :])
                            continue
                        rp = rpool.tile([128, nh, 128], f32r, tag="rope")
                        ev = ps3[:, :, 0:128:2]
                        od = ps3[:, :, 1:128:2]
                        cb = cos_t[:, None, sb, :].broadcast_to([128, nh, 64])
                        sn = sin_t[:, None, sb, :].broadcast_to([128, nh, 64])
                        t1 = rpool.tile([128, nh, 64], f32, tag="t1")
                        t2 = rpool.tile([128, nh, 64], f32, tag="t2")
                        nc.vector.tensor_tensor(out=t1, in0=ev, in1=cb, op=ALU.mult)
                        nc.vector.tensor_tensor(out=t2, in0=od, in1=sn, op=ALU.mult)
                        nc.vector.tensor_tensor(
                            out=rp[:, :, 0:64], in0=t1, in1=t2, op=ALU.subtract)
                        nc.vector.tensor_tensor(out=t1, in0=ev, in1=sn, op=ALU.mult)
                        nc.vector.tensor_tensor(out=t2, in0=od, in1=cb, op=ALU.mult)
                        nc.vector.tensor_tensor(
                            out=rp[:, :, 64:128], in0=t1, in1=t2, op=ALU.add)
                        for h in range(nh):
                            pt = ptp.tile([128, 128], f32r, tag="pt")
                            nc.tensor.transpose(pt, rp[:, h, :], ident)
                            dst = (qT[head_base + h] if kind == "q"
                                   else kT[head_base + h])
                            nc.vector.tensor_copy(
                                out=dst[:, sb * 128:(sb + 1) * 128], in_=pt)

            proj_pass(wkT[:, :], GPC * HD, "k")
            proj_pass(wvT[:, :], GPC * HD, "v")
            proj_pass(wqT[:, 0:512], 512, "q", head_base=0)
            proj_pass(wqT[:, 512:1024], 512, "q", head_base=4)

            # ------------ Stage 2+3: attention + out-projection -------------
            with tc.tile_pool(name="wo2", bufs=3) as wopool, \
                 tc.tile_pool(name="pr2", bufs=1) as prpool, \
                 tc.tile_pool(name="pts2", bufs=3) as ptsb_pool, \
                 tc.tile_pool(name="att2", bufs=1) as attpool, \
                 tc.tile_pool(name="dn2", bufs=2) as dnpool, \
                 tc.tile_pool(name="o2", bufs=3) as opool, \
                 tc.tile_pool(name="psc", bufs=2, space="PSUM") as pscp, \
                 tc.tile_pool(name="ppt", bufs=2, space="PSUM") as pptp, \
                 tc.tile_pool(name="pav", bufs=2, space="PSUM") as pavp, \
                 tc.tile_pool(name="pou", bufs=2, space="PSUM") as poup:
                for qsb in range(QSB):
                    att = attpool.tile([128, HPC, 512], f32r, tag="att")
                    if causal:
                        ext = [qsb * 512 + (j + 1) * 128 for j in range(4)]
                    else:
                        ext = [S] * 4
                    nkt = [e // 128 for e in ext]
                    maxkt = nkt[3]
                    for g in range(GPC):
                        for r in range(NREP):
                            h = g * NREP + r
                            probs = prpool.tile([128, 4, S], f32r, tag="probs")
                            dns = dnpool.tile([128, 4, 5], f32, tag="dns")
                            nc.vector.memset(dns, 0.0)
                            for j in range(4):
                                q0 = qsb * 512 + j * 128
                                nch = (ext[j] + 511) // 512
                                for ci in range(nch):
                                    k0 = ci * 512
                                    kw = min(512, ext[j] - k0)
                                    sc = pscp.tile([128, 512], f32, tag="sc")
                                    nc.tensor.matmul(
                                        sc[:, 0:kw], qT[h][:, q0:q0 + 128],
                                        kT[g][:, k0:k0 + kw],
                                        start=True, stop=True)
                                    is_diag = causal and (k0 + kw == ext[j])
                                    if not is_diag:
                                        nc.scalar.activation(
                                            out=probs[:, j, k0:k0 + kw],
                                            in_=sc[:, 0:kw], func=AF.Exp,
                                            scale=SCALE,
                                            accum_out=dns[:, j, ci:ci + 1])
                                    else:
                                        d0 = kw - 128
                                        nc.vector.scalar_tensor_tensor(
                                            out=sc[:, d0:kw], in0=sc[:, d0:kw],
                                            scalar=SCALE, in1=msk,
                                            op0=ALU.mult, op1=ALU.add)
                                        if d0 > 0:
                                            nc.scalar.activation(
                                                out=probs[:, j, k0:k0 + d0],
                                                in_=sc[:, 0:d0], func=AF.Exp,
                                                scale=SCALE,
                                                accum_out=dns[:, j, ci:ci + 1])
                                            nc.scalar.activation(
                                                out=probs[:, j, k0 + d0:k0 + kw],
                                                in_=sc[:, d0:kw], func=AF.Exp,
                                                scale=1.0,
                                                accum_out=dns[:, j, 4:5])
                                            nc.vector.tensor_tensor(
                                                out=dns[:, j, ci:ci + 1],
                                                in0=dns[:, j, ci:ci + 1],
                                                in1=dns[:, j, 4:5], op=ALU.add)
                                        else:
                                            nc.scalar.activation(
                                                out=probs[:, j, k0:k0 + kw],
                                                in_=sc[:, d0:kw], func=AF.Exp,
                                                scale=1.0,
                                                accum_out=dns[:, j, ci:ci + 1])
                            den = dnpool.tile([128, 4], f32, tag="den")
                            nc.vector.reduce_sum(
                                out=den, in_=dns[:, :, 0:4],
                                axis=mybir.AxisListType.X)
                            rec = dnpool.tile([128, 4], f32, tag="rec")
                            nc.vector.reciprocal(out=rec, in_=den)
                            for j in range(4):
                                sl = probs[:, j, 0:ext[j]]
                                if h % 2 == 0:
                                    nc.vector.tensor_scalar(
                                        out=sl, in0=sl,
                                        scalar1=rec[:, j:j + 1], scalar2=None,
                                        op0=ALU.mult)
                                else:
                                    nc.scalar.mul(sl, sl, rec[:, j:j + 1])
                            # AV over k-tiles
                            av = pavp.tile([128, 512], f32, tag="av")
                            for t in range(maxkt):
                                jlo = 0
                                while causal and nkt[jlo] <= t:
                                    jlo += 1
                                lo = jlo * 128
                                ptps = pptp.tile([128, 512], f32r, tag="ptps")
                                for j in range(jlo, 4):
                                    nc.tensor.matmul(
                                        ptps[:, j * 128:(j + 1) * 128],
                                        probs[:, j, t * 128:(t + 1) * 128],
                                        ident,
                                        is_transpose=True,
                                        start=True, stop=True)
                                pts = ptsb_pool.tile([128, 512], f32r, tag="pts")
                                if t % 3 == 2:
                                    nc.scalar.copy(
                                        out=pts[:, lo:512], in_=ptps[:, lo:512])
                                else:
                                    nc.vector.tensor_copy(
                                        out=pts[:, lo:512], in_=ptps[:, lo:512])
                                nc.tensor.matmul(
                                    av[:, lo:512], vsb[g][:, t, :],
                                    pts[:, lo:512],
                                    start=(t == 0), stop=(t == maxkt - 1),
                                    skip_group_check=True)
                            nc.vector.tensor_copy(out=att[:, h, :], in_=av)
                    # out-projection for this q-superblock
                    for m in range(KT):
                        wom = wopool.tile([128, HPC, 128], f32r, tag="wom")
                        nc.sync.dma_start(
                            out=wom, in_=woT3[:, :, m * 128:(m + 1) * 128])
                        po = poup.tile([128, 512], f32, tag="po")
                        for e in range(HPC):
                            nc.tensor.matmul(
                                po, wom[:, e, :], att[:, e, :],
                                start=(e == 0), stop=(e == HPC - 1))
                        ot = opool.tile([128, 512], f32, tag="ot")
                        nc.scalar.copy(out=ot, in_=po)
                        nc.sync.dma_start(
                            out=outT[m * 128:(m + 1) * 128,
                                     qsb * 512:(qsb + 1) * 512],
                            in_=ot)

    nc.compile()
    return nc


def _get_nc(causal: bool):
    if causal not in _compiled:
        _compiled[causal] = _build(causal)
    return _compiled[causal]


def kernel(x, freqs_cis, mask, wq, wk, wv, wo):
    from concourse.bass_utils import run_bass_kernel_spmd

    x = np.asarray(x, dtype=np.float32)
    freqs_cis = np.asarray(freqs_cis, dtype=np.float32)
    mask = np.asarray(mask, dtype=np.float32)
    wq = np.asarray(wq, dtype=np.float32)
    wk = np.asarray(wk, dtype=np.float32)
    wv = np.asarray(wv, dtype=np.float32)
    wo = np.asarray(wo, dtype=np.float32)

    tri = np.tril(np.ones((S, S), dtype=bool))
    causal = bool((mask[tri] == 0.0).all() and (mask[~tri] < -1e30).all())
    if not causal and not (mask == 0.0).all():
        return _numpy_ref(x, freqs_cis, mask, wq, wk, wv, wo)

    nc = _get_nc(causal)

    cos = freqs_cis[:, :, 0]
    sin = freqs_cis[:, :, 1]
    cosS = np.ascontiguousarray(cos.reshape(SB, 128, 64).transpose(1, 0, 2))
    sinS = np.ascontiguousarray(sin.reshape(SB, 128, 64).transpose(1, 0, 2))
    mtile = (np.ascontiguousarray(mask[0:128, 0:128]) if causal
             else np.zeros((128, 128), dtype=np.float32))

    in_maps = []
    for c in range(8):
        b, i = c // 2, c % 2
        in_maps.append({
            "xT": np.ascontiguousarray(x[b].T),
            "wqT": np.ascontiguousarray(wq[1024 * i:1024 * (i + 1), :].T),
            "wkT": np.ascontiguousarray(wk[256 * i:256 * (i + 1), :].T),
            "wvT": np.ascontiguousarray(wv[256 * i:256 * (i + 1), :].T),
            "woT": np.ascontiguousarray(wo[:, 1024 * i:1024 * (i + 1)].T),
            "cosS": cosS, "sinS": sinS, "mtile": mtile,
        })

    res = run_bass_kernel_spmd(nc, in_maps, core_ids=list(range(8)))
    out = np.empty((B, S, D), dtype=np.float32)
    for b in range(B):
        out[b] = res.results[2 * b]["outT"].T + res.results[2 * b + 1]["outT"].T
    return out


def _numpy_ref(x, freqs_cis, mask, wq, wk, wv, wo):
    xq = (x @ wq.T).reshape(B, S, H, HD)
    xk = (x @ wk.T).reshape(B, S, KV, HD)
    xv = (x @ wv.T).reshape(B, S, KV, HD)

    def rope(xh):
        x2 = xh.reshape(*xh.shape[:-1], HD // 2, 2)
        fc = freqs_cis[None, :, None, :, :]
        real = x2[..., 0] * fc[..., 0] - x2[..., 1] * fc[..., 1]
        imag = x2[..., 0] * fc[..., 1] + x2[..., 1] * fc[..., 0]
        return np.concatenate([real, imag], axis=-1)

    xq, xk = rope(xq), rope(xk)
    q = xq.reshape(B, S, KV, NREP, HD)
    sc = np.einsum('bqgrd,bkgd->bgrqk', q, xk) * SCALE + mask[None, None, None]
    sc = sc - sc.max(axis=-1, keepdims=True)
    p = np.exp(sc)
    p /= p.sum(axis=-1, keepdims=True)
    o = np.einsum('bgrqk,bkgd->bqgrd', p, xv).reshape(B, S, H * HD)
    return (o @ wo.T).astype(np.float32)
